# revision 1
# baseline (speedup 1.0000x reference)
"""Multi-head attention (2-axis RoPE) Trainium2 kernel, 8-core data parallel.

Problem (hardcoded): B=16, S=1024 (32x32 grid), E=256, H=8, D=32, fp32 I/O.
  qkv = x @ Wqkv + bqkv ; RoPE(q), RoPE(k) ; softmax(q k^T / sqrt(D)) @ v ; @ Wout + bout

Sharding: batch across 8 cores (2 batches/core). Host scatters inputs /
gathers outputs; each core runs the full attention for its 2 batches.

v3 design notes (per core, T=2048 tokens):
  - all matmul operands bf16 (fp32 rhs streams at 2 cyc/col, bf16 at 1).
  - warmup: dummy matmuls at t=0 keep the PE HAM un-throttled through the
    initial DMA window (cold PE runs at 1.2 instead of 2.4 GHz).
  - rope: qkv proj -> ScalarE casts PSUM->bf16, DVE does both cos/sin muls
    in 2x bf16 mode, PE pair-swap permute, DVE add.  cos/sin tables are
    [128, S] bf16 (token range repeats per batch).
  - scores transposed [sk, sq], 4 heads packed as 32-row-band concurrent
    matmuls; exp split ScalarE (table exp) / VectorE (custom EXP16 op,
    one 8-stage instruction: ((z*c0 + c1)^2 + 0.5)^16, rel err ~1.5e-4).
  - AV: 4 heads as concurrent col-tiled M=32 matmuls; a parallel block-ones
    matmul accumulates the softmax denominator pre-broadcast; one
    reciprocal_approx_fast + one tensor_mul normalizes 4 heads at once.
  - software-pipelined emission: AV/den groups lag scores by 2 tiles and
    cross iteration boundaries so neither exp engine stalls at seams.
"""

import math

import numpy as np

B, G, H, D, E = 16, 32, 8, 32, 256
S = G * G
NCORES = 8
B_LOC = B // NCORES
T = B_LOC * S  # tokens per core
SCALE = 1.0 / math.sqrt(D)

# EXP16 constants: exp(x*SCALE) ~= ((x*C0p + C1p)^2 + C2p)^16
EXP_N = 16
C0P = SCALE / (EXP_N * math.sqrt(2.0))
C1P = 1.0 / math.sqrt(2.0)
C2P = 0.5

# which of the 16 half-tiles (t = 2*j + p) per (b, g, half) iteration are
# exp'd on the DVE (rest on ScalarE); alternate so both engines stay fed
DVE_TILES = frozenset((1, 3, 5, 7, 9, 11, 13))

N_WARMUP = 220  # dummy PE matmuls issued at t=0 (HAM warmup during DMA)

_COMPILED = None
_LAST_RESULT = None  # test.py reads exec_time_ns / trace path from here


def _bf16(a):
    import ml_dtypes

    return np.asarray(a).astype(ml_dtypes.bfloat16)


def _rope_tables():
    """cos/sin [128, S] feature-major (row p multiplies feature d = p % 32 of
    every head; identical for every batch).  Device computes
    rope(x) = x*cos + P(x*sin) with P the pair swap (p ^ 1); the sign
    pattern sits pre-permutation: even rows +sin, odd rows -sin."""
    freqs = 1.0 / (10000.0 ** (np.arange(0, D, 4, dtype=np.float64) / D))  # [8]
    t = np.arange(G, dtype=np.float64)
    fx = t[:, None] * freqs[None, :]  # [32, 8]
    ax = np.broadcast_to(fx[:, None, :], (G, G, D // 4))
    ay = np.broadcast_to(fx[None, :, :], (G, G, D // 4))
    ang = np.concatenate([ax, ay], axis=-1).reshape(S, D // 2)  # [1024, 16]
    cos = np.cos(ang).astype(np.float32)  # [S, 16]
    sin = np.sin(ang).astype(np.float32)
    p = np.arange(128)
    pair = (p % D) // 2  # [128]
    sgn = np.where(p % 2 == 0, 1.0, -1.0).astype(np.float32)
    cosT = np.ascontiguousarray(cos[:, pair].T)  # [128, S]
    sinT = np.ascontiguousarray(sin[:, pair].T * sgn[:, None])
    return cosT, sinT


def _exp16_ref(in0, in1, c0, c1, c2):
    x = in0.astype(np.float32)
    u = (x * np.float32(c0) + np.float32(c1)).astype(np.float32)
    u = (u * u + np.float32(c2)).astype(np.float32)
    for _ in range(4):
        u = (u * u).astype(np.float32)
    return u


def _register_exp16():
    """Register the EXP16 custom DVE op (one 8-stage instruction) in
    concourse.dve_ops so the per-NEFF table generator and CoreSim see it."""
    import concourse.dve_ops as dops
    from concourse.dve_spec import C0, C1, C2, Spec, Src0, lower, sq
    from concourse.dve_uop import DveOpSpec

    name = "EXP16_MHA"
    for o in dops.OPS:
        if o.name == name:
            return o
    u = sq(Src0 * C0 + C1) + C2
    for _ in range(4):
        u = sq(u)
    spec = Spec(body=u, reference=_exp16_ref)
    row = max(dops._SUB_OPCODE_FOR_NAME.values()) + 1
    assert row < 0x20
    shas = {}
    for ver in ("v3", "v4"):
        shas[ver] = DveOpSpec(
            name=name, opcode=row, uops=lower(spec, ver=ver), rd1_en=False
        ).sha(ver)
    op = dops.DveOp(name, spec, subdim=False, uops_sha=shas)
    dops.OPS.append(op)
    dops._SUB_OPCODE_FOR_NAME[name] = row
    dops.CUSTOM_DVE_SPECS[name] = spec
    return op


def _build():
    import concourse.bass as bass  # noqa: F401
    import concourse.tile as tile
    from concourse import bacc, mybir

    f32 = mybir.dt.float32
    bf16 = mybir.dt.bfloat16
    exp16_op = _register_exp16()

    nc = bacc.Bacc("TRN2", target_bir_lowering=False, debug=False, num_devices=NCORES)

    xT_d = nc.dram_tensor("xT_aug", [E + 1, T], bf16, kind="ExternalInput").ap()
    wqk_d = nc.dram_tensor("wqk_aug", [E + 1, 2 * E], bf16, kind="ExternalInput").ap()
    wv_d = nc.dram_tensor("wv_aug", [E + 1, E], bf16, kind="ExternalInput").ap()
    wo_d = nc.dram_tensor("wo_aug", [E + 1, E], bf16, kind="ExternalInput").ap()
    cos_d = nc.dram_tensor("cosT", [128, S], bf16, kind="ExternalInput").ap()
    sin_d = nc.dram_tensor("sinT", [128, S], bf16, kind="ExternalInput").ap()
    psw_d = nc.dram_tensor("pswap", [128, 128], bf16, kind="ExternalInput").ap()
    qkb_d = nc.dram_tensor("qkbT", [128, 4], f32, kind="ExternalInput").ap()
    out_d = nc.dram_tensor("out", [T, E], f32, kind="ExternalOutput").ap()

    with tile.TileContext(nc) as tc:
        consts = tc.alloc_tile_pool(name="consts", bufs=1)
        work = tc.alloc_tile_pool(name="work", bufs=1)

        ones_blk = consts.tile([128, 32], bf16, name="ones_blk")
        nc.vector.memset(ones_blk, 1.0)

        # ---- constant / weight loads (priority order: proj deps first) ---
        xT_a = consts.tile([128, T], bf16, name="xT_a")
        xT_b = consts.tile([128, T], bf16, name="xT_b")
        xT_ones = consts.tile([1, T], bf16, name="xT_ones")
        wqk_a = consts.tile([128, 2 * E], bf16, name="wqk_a")
        wqk_b = consts.tile([128, 2 * E], bf16, name="wqk_b")
        qkbT = consts.tile([128, 4], f32, name="qkbT")
        cosT = consts.tile([128, S], bf16, name="cosT")
        sinT = consts.tile([128, S], bf16, name="sinT")
        pswap = consts.tile([128, 128], bf16, name="pswap")
        wv_a = consts.tile([128, E], bf16, name="wv_a")
        wv_b = consts.tile([128, E], bf16, name="wv_b")
        wv_c = consts.tile([1, E], bf16, name="wv_c")
        wo_a = consts.tile([128, E], bf16, name="wo_a")
        wo_b = consts.tile([128, E], bf16, name="wo_b")
        wo_c = consts.tile([1, E], bf16, name="wo_c")

        nc.sync.dma_start(out=xT_a, in_=xT_d[0:128, :])
        nc.sync.dma_start(out=wqk_a, in_=wqk_d[0:128, :])
        nc.sync.dma_start(out=xT_b, in_=xT_d[128:256, :])
        nc.sync.dma_start(out=wqk_b, in_=wqk_d[128:256, :])
        nc.sync.dma_start(out=xT_ones, in_=xT_d[256:257, :])
        nc.sync.dma_start(out=qkbT, in_=qkb_d)
        nc.sync.dma_start(out=sinT, in_=sin_d)
        nc.sync.dma_start(out=cosT, in_=cos_d)
        nc.sync.dma_start(out=pswap, in_=psw_d)
        nc.sync.dma_start(out=wv_a, in_=wv_d[0:128, :])
        nc.sync.dma_start(out=wv_b, in_=wv_d[128:256, :])
        nc.sync.dma_start(out=wv_c, in_=wv_d[256:257, :])
        nc.sync.dma_start(out=wo_a, in_=wo_d[0:128, :])
        nc.sync.dma_start(out=wo_b, in_=wo_d[128:256, :])
        nc.sync.dma_start(out=wo_c, in_=wo_d[256:257, :])

        xT_chunks = [xT_a, xT_b, xT_ones]
        wqk_chunks = [wqk_a, wqk_b]
        wv_chunks = [wv_a, wv_b, wv_c]
        wo_chunks = [wo_a, wo_b, wo_c]

        # feature-major roped q/k: 4 chunks of 128 rows (q heads 0-7, k 0-7)
        qk_rope = [
            consts.tile([128, T], bf16, name=f"qk_rope{m}", tag=f"qk_rope{m}")
            for m in range(4)
        ]
        # v token-major: [128 tok, tok_tile, head, 32] bf16
        v_all = consts.tile([128, T // 128, H, D], bf16, name="v_all")
        # attention output, feature-major bf16: 2 chunks of 128 rows
        att_oT = [
            consts.tile([128, T], bf16, name=f"att_oT{g}", tag=f"att_oT{g}")
            for g in range(2)
        ]

        # ================= phase 1: qk projection + rope, v projection ====
        with tc.tile_pool(name="ps1", bufs=1, space="PSUM") as ps1:
            # HAM warmup: keep the PE busy while input DMAs stream so real
            # matmuls start at 2.4 GHz instead of the cold 1.2 GHz.
            warm = ps1.tile([128, E], f32, name="v_ps", tag="v_ps", bufs=2)
            with nc.named_scope("warm"):
                for _ in range(N_WARMUP):
                    nc.tensor.matmul(
                        out=warm[0:32, 0:32], lhsT=ones_blk, rhs=ones_blk[:, 0:32],
                        start=True, stop=True,
                    )

            def v_proj(tt):
                with nc.named_scope("vproj"):
                    tsl = slice(tt * 128, (tt + 1) * 128)
                    v_ps = ps1.tile([128, E], f32, name="v_ps", tag="v_ps", bufs=2)
                    for k in range(3):
                        nc.tensor.matmul(
                            out=v_ps,
                            lhsT=xT_chunks[k][:, tsl],
                            rhs=wv_chunks[k],
                            start=(k == 0),
                            stop=(k == 2),
                        )
                    # cast fp32 PSUM -> bf16 SBUF on ScalarE
                    nc.scalar.copy(
                        out=v_all[:, tt, :, :].rearrange("p h d -> p (h d)"),
                        in_=v_ps,
                    )

            def rope_tail(m, sl, qk_bf, t_sin):
                with nc.named_scope("rope"):
                    perm_ps = ps1.tile(
                        [128, 1024], f32, name="perm_ps", tag="perm_ps", bufs=1
                    )
                    for hv in range(2):
                        osl = slice(hv * 512, hv * 512 + 512)
                        nc.tensor.matmul(
                            out=perm_ps[:, osl], lhsT=pswap, rhs=t_sin[:, osl],
                            start=True, stop=True,
                        )
                    t_cos = work.tile([128, 1024], bf16, name="t_cos",
                                      tag="t_cos", bufs=2)
                    nc.vector.tensor_mul(t_cos, qk_bf, cosT)  # 2x bf16
                    nc.vector.tensor_add(qk_rope[m][:, sl], t_cos, perm_ps)

            unit = 0
            pend1 = []
            for m in range(4):
                for n in range(2):  # 1024-col slices over tokens
                    sl = slice(n * 1024, n * 1024 + 1024)
                    qk_ps = ps1.tile(
                        [128, 1024], f32, name="qk_ps", tag="qk_ps", bufs=2
                    )
                    with nc.named_scope("qkproj"):
                        for hv in range(2):  # N=512 matmul/PSUM-bank limit
                            osl = slice(hv * 512, hv * 512 + 512)
                            xsl = slice(
                                n * 1024 + hv * 512, n * 1024 + hv * 512 + 512
                            )
                            for k in range(2):
                                nc.tensor.matmul(
                                    out=qk_ps[:, osl],
                                    lhsT=wqk_chunks[k][:, m * 128 : (m + 1) * 128],
                                    rhs=xT_chunks[k][:, xsl],
                                    start=(k == 0),
                                    stop=(k == 1),
                                )
                    if pend1:
                        pend1.pop(0)()
                    with nc.named_scope("rope"):
                        qk_bf = work.tile([128, 1024], bf16, name="qk_bf",
                                          tag="qk_bf", bufs=3)
                        # cast + per-feature qkv bias in one ScalarE pass
                        nc.scalar.add(out=qk_bf, in_=qk_ps,
                                      add=qkbT[:, m : m + 1])
                        t_sin = work.tile([128, 1024], bf16, name="t_sin",
                                          tag="t_sin", bufs=2)
                        nc.vector.tensor_mul(t_sin, qk_bf, sinT)  # 2x bf16
                    pend1.append(
                        lambda mm=m, ss=sl, qb=qk_bf, ts=t_sin: rope_tail(
                            mm, ss, qb, ts
                        )
                    )
                    # two v-proj tiles per unit keep the PE dense (and warm)
                    v_proj(unit * 2)
                    v_proj(unit * 2 + 1)
                    unit += 1
            for f in pend1:
                f()

        # ================= phase 2: attention (software-pipelined) ========
        with tc.tile_pool(name="ps2", bufs=1, space="PSUM") as ps2:
            pending = []  # deferred emission closures (av/den units, norms)

            def drain_to(nmax, max_pop=2):
                popped = 0
                while len(pending) > nmax and popped < max_pop:
                    pending.pop(0)()
                    popped += 1

            def make_iter(b, g, half):
                qc = qk_rope[g]
                kc = qk_rope[2 + g]
                qsl = slice(b * S + half * 512, b * S + half * 512 + 512)
                o_ps = ps2.tile([128, 512], f32, name="o_ps", tag="o_ps", bufs=1)
                den_ps = ps2.tile(
                    [128, 512], f32, name="den_ps", tag="den_ps", bufs=1
                )
                tiles = {}

                def score_exp(j):
                    ksl = slice(b * S + j * 128, b * S + j * 128 + 128)
                    for p in range(2):
                        s_ps = ps2.tile(
                            [128, 2, 512], f32, name="s_ps", tag="s_ps", bufs=3
                        )
                        with nc.named_scope("score"):
                            for e in range(2):
                                hl = 2 * p + e
                                psl = slice(32 * hl, 32 * hl + 32)
                                nc.tensor.matmul(
                                    out=s_ps[:, e, :],
                                    lhsT=kc[psl, ksl],
                                    rhs=qc[psl, qsl],
                                    start=True,
                                    stop=True,
                                    tile_position=(32 * hl, 0),
                                )
                        at = work.tile(
                            [128, 2, 512], bf16, name="attn", tag="attn", bufs=10
                        )
                        if 2 * j + p in DVE_TILES:
                            with nc.named_scope("exp_dve"):
                                nc.vector._custom_dve(
                                    exp16_op, out=at, in0=s_ps,
                                    s0=C0P, s1=C1P, imm2=C2P,
                                )
                        else:
                            with nc.named_scope("exp_sc"):
                                nc.scalar.activation(
                                    out=at,
                                    in_=s_ps,
                                    func=mybir.ActivationFunctionType.Exp,
                                    scale=SCALE,
                                )
                        tiles[(j, p)] = at

                def av_den(j):
                    # 4 AV matmuls (distinct col groups -> concurrent), then
                    # 4 den matmuls; interleaving would serialize col groups.
                    with nc.named_scope("av"):
                        for p in range(2):
                            at = tiles[(j, p)]
                            for e in range(2):
                                hl = 2 * p + e
                                osl = slice(32 * hl, 32 * hl + 32)
                                nc.tensor.matmul(
                                    out=o_ps[osl, :],
                                    lhsT=v_all[:, b * 8 + j, 4 * g + hl, :],
                                    rhs=at[:, e, :],
                                    start=(j == 0),
                                    stop=(j == 7),
                                    tile_position=(0, 32 * hl),
                                )
                    with nc.named_scope("den"):
                        for p in range(2):
                            at = tiles[(j, p)]
                            for e in range(2):
                                hl = 2 * p + e
                                osl = slice(32 * hl, 32 * hl + 32)
                                nc.tensor.matmul(
                                    out=den_ps[osl, :],
                                    lhsT=ones_blk,
                                    rhs=at[:, e, :],
                                    start=(j == 0),
                                    stop=(j == 7),
                                    tile_position=(0, 32 * hl),
                                )

                def norm():
                    with nc.named_scope("norm"):
                        bc = work.tile(
                            [128, 512], f32, name="bc", tag="bc", bufs=2
                        )
                        nc.vector.reciprocal_approx_fast(out=bc, in_=den_ps)
                        nc.vector.tensor_mul(att_oT[g][:, qsl], o_ps, bc)

                return score_exp, av_den, norm

            def out_proj_tile(tt):
                """One out-projection token tile; borrows an s_ps PSUM
                buffer so it can interleave with attention iterations."""
                with nc.named_scope("outproj"):
                    tsl = slice(tt * 128, (tt + 1) * 128)
                    f = ps2.tile(
                        [128, 2, 512], f32, name="f_ps", tag="s_ps", bufs=3
                    )
                    f_ps = f[:, 0, 0:E]
                    for k in range(3):
                        lhsT = (att_oT[0], att_oT[1], xT_ones)[k][:, tsl]
                        nc.tensor.matmul(
                            out=f_ps,
                            lhsT=lhsT,
                            rhs=wo_chunks[k],
                            start=(k == 0),
                            stop=(k == 2),
                        )
                    o_sb = work.tile(
                        [128, E], f32, name="o_sb", tag="o_sb", bufs=4
                    )
                    if tt % 2 == 0:
                        nc.scalar.copy(out=o_sb, in_=f_ps)
                    else:
                        nc.vector.tensor_copy(out=o_sb, in_=f_ps)
                    nc.sync.dma_start(out=out_d[tsl, :], in_=o_sb)

            for b in range(B_LOC):
                for g in range(2):
                    for half in range(2):
                        score_exp, av_den, norm = make_iter(b, g, half)
                        for j in range(8):
                            score_exp(j)
                            pending.append(
                                (lambda f=av_den, jj=j: f(jj))
                            )
                            drain_to(2)
                        pending.append(norm)
                        # queue out-proj tiles as soon as their att_oT
                        # columns are complete (token tile tt needs both
                        # head groups of its half)
                        if b == 1 and g == 1 and half == 0:
                            for tt in range(8, 12):
                                pending.append(lambda t=tt: out_proj_tile(t))
                        if g == 1 and half == 1:
                            rng = range(0, 8) if b == 0 else range(12, 16)
                            for tt in rng:
                                pending.append(lambda t=tt: out_proj_tile(t))
            while pending:
                pending.pop(0)()

        work.release()
        consts.release()

    nc.compile()
    return nc


def _prep_core_inputs(x_loc, Wqkv, bqkv, Wout, bout, cosT, sinT, pswap):
    xT = x_loc.reshape(T, E).T.astype(np.float32)  # [256, T]
    xT_aug = np.concatenate([xT, np.ones((1, T), np.float32)], axis=0)
    wqk_aug = np.concatenate([Wqkv[:, : 2 * E], bqkv[None, : 2 * E]], axis=0)
    wv_aug = np.concatenate([Wqkv[:, 2 * E :], bqkv[None, 2 * E :]], axis=0)
    wo_aug = np.concatenate([Wout, bout[None, :]], axis=0)
    qkbT = np.ascontiguousarray(
        bqkv[: 2 * E].reshape(4, 128).T.astype(np.float32)
    )  # column m = bias for qk feature chunk m
    return {
        "xT_aug": np.ascontiguousarray(_bf16(xT_aug)),
        "wqk_aug": np.ascontiguousarray(_bf16(wqk_aug)),
        "wv_aug": np.ascontiguousarray(_bf16(wv_aug)),
        "wo_aug": np.ascontiguousarray(_bf16(wo_aug)),
        "cosT": np.ascontiguousarray(_bf16(cosT)),
        "sinT": np.ascontiguousarray(_bf16(sinT)),
        "pswap": _bf16(pswap),
        "qkbT": qkbT,
    }


def _pswap_mat():
    p = np.zeros((128, 128), np.float32)
    idx = np.arange(128)
    p[idx, idx ^ 1] = 1.0
    return p


def kernel(x, Wqkv, bqkv, Wout, bout):
    global _COMPILED, _LAST_RESULT
    from concourse.bass_utils import run_bass_kernel_spmd

    if _COMPILED is None:
        _COMPILED = _build()
    nc = _COMPILED

    x = np.asarray(x, np.float32)
    Wqkv = np.asarray(Wqkv, np.float32)
    bqkv = np.asarray(bqkv, np.float32)
    Wout = np.asarray(Wout, np.float32)
    bout = np.asarray(bout, np.float32)

    cosT, sinT = _rope_tables()
    pswap = _pswap_mat()

    in_maps = [
        _prep_core_inputs(
            x[c * B_LOC : (c + 1) * B_LOC], Wqkv, bqkv, Wout, bout, cosT, sinT, pswap
        )
        for c in range(NCORES)
    ]
    res = run_bass_kernel_spmd(nc, in_maps, list(range(NCORES)))
    _LAST_RESULT = res
    out = np.stack([res.results[c]["out"].reshape(B_LOC, S, E) for c in range(NCORES)])
    return np.ascontiguousarray(out.reshape(B, S, E))


# ---------------------------------------------------------------------------
# host model: numpy mirror of the device dataflow (bf16 casts, EXP16 tiles)
def host_model(x, Wqkv, bqkv, Wout, bout):
    def f32(a):
        return np.asarray(a, np.float32)

    cosT, sinT = _rope_tables()
    cosT_b = f32(_bf16(cosT))
    sinT_b = f32(_bf16(sinT))
    perm = np.arange(128) ^ 1
    outs = []
    for c in range(NCORES):
        m = _prep_core_inputs(
            f32(x)[c * B_LOC : (c + 1) * B_LOC], f32(Wqkv), f32(bqkv), f32(Wout),
            f32(bout), cosT, sinT, _pswap_mat(),
        )
        xT_aug = f32(m["xT_aug"])
        wqk, wv, wo = f32(m["wqk_aug"]), f32(m["wv_aug"]), f32(m["wo_aug"])
        qkb = f32(m["qkbT"])
        qkT = wqk[:256].T @ xT_aug[:256]  # [512, T] fp32 accum of bf16 operands
        qkr = np.empty((512, T), np.float32)
        cs2 = np.tile(cosT_b, (1, B_LOC))
        sn2 = np.tile(sinT_b, (1, B_LOC))
        for mm in range(4):
            # ScalarE cast + per-feature bias
            ch = f32(_bf16(qkT[mm * 128 : (mm + 1) * 128] + qkb[:, mm : mm + 1]))
            t_sin = f32(_bf16(ch * sn2))
            t_cos = f32(_bf16(ch * cs2))
            qkr[mm * 128 : (mm + 1) * 128] = f32(_bf16(t_cos + t_sin[perm, :]))
        v = f32(_bf16((xT_aug.T @ wv))).reshape(T, H, D)  # token-major bf16
        att_oT = np.empty((256, T), np.float32)
        for b in range(B_LOC):
            for g in range(2):
                for half in range(2):
                    qsl = slice(b * S + half * 512, b * S + half * 512 + 512)
                    o_acc = np.zeros((128, 512), np.float32)
                    den_acc = np.zeros((4, 512), np.float32)
                    for j in range(8):
                        ksl = slice(b * S + j * 128, b * S + j * 128 + 128)
                        for p in range(2):
                            for e in range(2):
                                hl = 2 * p + e
                                psl = slice(g * 128 + 32 * hl, g * 128 + 32 * hl + 32)
                                kc = qkr[256 + psl.start : 256 + psl.stop, ksl]
                                qc = qkr[psl, qsl]
                                scores = kc.T @ qc  # [128, 512]
                                if 2 * j + p in DVE_TILES:
                                    ex = _exp16_ref(scores, None, C0P, C1P, C2P)
                                else:
                                    ex = np.exp(scores * SCALE)
                                ex = f32(_bf16(ex))
                                vb = v[b * S + j * 128 : b * S + (j + 1) * 128,
                                       4 * g + hl]  # [128, 32]
                                o_acc[32 * hl : 32 * hl + 32] += vb.T @ ex
                                den_acc[hl] += ex.sum(0)
                    bc = 1.0 / den_acc  # recip_approx ~ exact here
                    o_n = np.empty_like(o_acc)
                    for hl in range(4):
                        o_n[32 * hl : 32 * hl + 32] = (
                            o_acc[32 * hl : 32 * hl + 32] * bc[hl]
                        )
                    att_oT[g * 128 : (g + 1) * 128, qsl] = f32(_bf16(o_n))
        att_aug = np.concatenate(
            [f32(_bf16(att_oT)), np.ones((1, T), np.float32)], 0
        )
        out = att_aug.T @ wo
        outs.append(out.reshape(B_LOC, S, E))
    return np.concatenate(outs, 0).astype(np.float32)

